# revision 1
# baseline (speedup 1.0000x reference)
"""BLEU-precision loss kernel for Trainium2 (8 NeuronCores, data parallel).

loss = 1 - mean_i |set(pred_i) & set(tgt_i)| / |set(pred_i)|   over 64 rows.

Per core (8 rows): decompose each token t < 32000 into digits
lo = t & 127 (< 128) and b = t >> 7 (< 250). For each row and side, two
fp16 DVE is_equal ops build the digit one-hots in m-major tiles
oh[p, m*16 + f] = (dig[p, f] == m) against prebuilt iota tiles (dense
step-1 APs keep the DVE in 2x mode). The PE
accumulates the vocab count grid G[lo, b] = sum_k oh_lo(k)^T oh_b(k) over 16
chunks of 128 tokens per row (PSUM f32, exact). A vocab id is present iff
G > 0:
    setsize = sum(G_pred > 0),   overlap = sum((G_pred > 0) * (G_tgt > 0)).
Presence/reduce tails are interleaved per row into the one-hot/matmul
pipeline. Per-(partition,row) partial sums go to the host, which finishes
the tiny exact integer reductions and the final mean.
"""
import sys

sys.path.insert(0, "/opt/trn_rl_repo")

import numpy as np
import concourse.bass as bass
import concourse.bacc as bacc
from concourse import mybir

B = 64          # total rows
ROWS = 8        # rows per core
N_CORES = 8
L = 2048        # tokens per row
V = 32000       # vocab
P = 128
NB = 250        # b-digit range
W = P + NB      # combined one-hot width per token (378)
GSTRIDE = 256   # f32 row stride inside the PSUM grid (1KB, bank aligned)
CH = 16         # chunks of 128 tokens per row
F = mybir.dt.float32
F16 = mybir.dt.float16
I32 = mybir.dt.int32

_CACHE = {}


def _build_kernel():
    nc = bacc.Bacc()
    pred = nc.dram_tensor("pred", [ROWS, L], I32, kind="ExternalInput")
    tgt = nc.dram_tensor("tgt", [ROWS, L], I32, kind="ExternalInput")
    out = nc.dram_tensor("out", [P, 16], F, kind="ExternalOutput")

    # SBUF token layout per side: partition = (token index within row)//16,
    # free = row*16 + (token index)%16
    def tok_src(t):
        a = t[:]
        return bass.AP(a.tensor, 0, [[16, P], [L, ROWS], [1, 16]])

    from contextlib import ExitStack

    es = ExitStack()
    with es:
        sb = lambda name, shape, dt: es.enter_context(nc.sbuf_tensor(name, shape, dt))
        ps = lambda name, shape, dt: es.enter_context(nc.psum_tensor(name, shape, dt))
        sem = lambda name: es.enter_context(nc.semaphore(name))
        tok_p = sb("tok_p", [P, P], I32)
        tok_t = sb("tok_t", [P, P], I32)
        lo_i = sb("lo_i", [P, P], I32)
        b_i = sb("b_i", [P, P], I32)
        # per side: cols 16r..16r+16 = row r's lo digits
        lo_p = sb("lo_p", [P, P], F16)
        bs_p = sb("bs_p", [P, P], F16)
        lo_t = sb("lo_t", [P, P], F16)
        bs_t = sb("bs_t", [P, P], F16)
        iota_c = sb("iota_c", [P, CH * W], F16)
        oh0 = sb("oh0", [P, CH * W], F16)
        oh1 = sb("oh1", [P, CH * W], F16)
        oh2 = sb("oh2", [P, CH * W], F16)
        pres_p = sb("pres_p", [P, ROWS * NB], F16)
        pres_t = sb("pres_t", [P, ROWS * NB], F16)
        junk = sb("junk", [P, ROWS * NB], F16)
        res = sb("res", [P, 16], F)
        # one 2KB PSUM bank per tensor. Rows 1..6: bank r holds that row's
        # pred grid at [0,250) and tgt grid at [256,506). Rows 0 and 7 are
        # split across banks 0 and 7 (pred in bank0, tgt in bank7) so the
        # row-7 pred Sign can run while the PE still accumulates the row-7
        # tgt chain.
        G_r = [ps(f"G{r}", [P, 2 * GSTRIDE], F) for r in range(ROWS)]

        def g_slot(r, side):
            if r == 0:
                return (G_r[0], 0) if side == "p" else (G_r[7], 0)
            if r == ROWS - 1:
                return (G_r[0], GSTRIDE) if side == "p" else (G_r[7], GSTRIDE)
            return (G_r[r], 0 if side == "p" else GSTRIDE)
        s_load_p = sem("s_load_p")
        s_load_t = sem("s_load_t")
        s_pool = sem("s_pool")
        s_dve = sem("s_dve")
        s_pe = sem("s_pe")
        s_act = sem("s_act")
        s_out = sem("s_out")
        block = es.enter_context(nc.Block())

        oh = (oh0, oh1, oh2)
        lo_d = {"p": lo_p, "t": lo_t}
        bs_d = {"p": bs_p, "t": bs_t}
        ticks = {"oh": [0] * (2 * ROWS), "oh_h1": [0] * (2 * ROWS),
                 "prod": [0] * ROWS}

        @block.gpsimd
        def _(g):
            # combined m-major iota: addr m*CH+f -> value m for m in [0,128)
            # (lo half) then [128,378) (bs half). fp16 exact for ints < 2048.
            g.iota(iota_c[:, :CH * P].rearrange("p (m f) -> p m f", f=CH),
                   pattern=[[1, P], [0, CH]], base=0, channel_multiplier=0,
                   allow_small_or_imprecise_dtypes=True).then_inc(s_pool, 1)
            HB = NB // 2
            g.iota(iota_c[:, CH * P:CH * (P + HB)].rearrange("p (m f) -> p m f", f=CH),
                   pattern=[[1, HB], [0, CH]], base=0, channel_multiplier=0,
                   allow_small_or_imprecise_dtypes=True).then_inc(s_pool, 1)
            g.iota(iota_c[:, CH * (P + HB):].rearrange("p (m f) -> p m f", f=CH),
                   pattern=[[1, NB - HB], [0, CH]], base=HB, channel_multiplier=0,
                   allow_small_or_imprecise_dtypes=True).then_inc(s_pool, 1)

        @block.vector
        def _(v):
            t = 0

            def inc(ins):
                nonlocal t
                t += 1
                return ins.then_inc(s_dve, 1)

            def digits(side, tok, c0, c1):
                # bitVec ops cannot cast on HW: compute in i32, cast via copy
                cs = slice(c0, c1)
                inc(v.tensor_scalar(out=lo_i[:, cs], in0=tok[:, cs], scalar1=127,
                                    scalar2=None,
                                    op0=mybir.AluOpType.bitwise_and))
                inc(v.tensor_scalar(out=b_i[:, cs], in0=tok[:, cs],
                                    scalar1=7, scalar2=None,
                                    op0=mybir.AluOpType.logical_shift_right))
                v.wait_ge(s_dve, t)
                inc(v.tensor_copy(lo_d[side][:, cs], lo_i[:, cs]))
                inc(v.tensor_copy(bs_d[side][:, cs], b_i[:, cs]))
                v.wait_ge(s_dve, t)

            def tail(r):
                nonlocal t
                # presence came from ACT (Sign); DVE only forms the product,
                # which ACT then reduces via Identity+accum. The LAST row's
                # tgt presence, product, and reduce all run on the DVE
                # itself (idle then) to skip ACT round-trips.
                sl = slice(NB * r, NB * (r + 1))
                if r == ROWS - 1:
                    gt, ot_ = g_slot(r, "t")
                    v.wait_ge(s_pe, 2 * ROWS)  # final chain landed
                    inc(v.tensor_scalar(out=pres_t[:, sl],
                                        in0=gt[:, ot_:ot_ + NB],
                                        scalar1=0.5, scalar2=None,
                                        op0=mybir.AluOpType.is_gt))
                    v.wait_ge(s_act, 3 * ROWS - 3)  # pres_p7 (ACT tick 21)
                    v.wait_ge(s_dve, t)
                    inc(v.tensor_tensor(out=junk[:, sl], in0=pres_p[:, sl],
                                        in1=pres_t[:, sl],
                                        op=mybir.AluOpType.mult))
                    v.wait_ge(s_dve, t)
                    v.reduce_sum(out=res[:, r:r + 1], in_=junk[:, sl],
                                 axis=mybir.AxisListType.X)
                    t += 1
                    v.drain().then_inc(s_dve, 1)
                    ticks["final"] = t
                    return
                v.wait_ge(s_act, 2 if r == 0 else 3 * r + 1)
                inc(v.tensor_tensor(out=junk[:, sl], in0=pres_p[:, sl],
                                    in1=pres_t[:, sl],
                                    op=mybir.AluOpType.mult))
                ticks["prod"][r] = t

            v.wait_ge(s_load_p, 16)
            digits("p", tok_p, 0, P)
            v.wait_ge(s_load_t, 16)
            digits("t", tok_t, 0, P)

            for rs in range(2 * ROWS):
                r, side = rs // 2, ("p", "t")[rs % 2]
                buf = rs % 3
                if rs == 0:
                    v.wait_ge(s_pool, 1)  # iota lo half ready
                if rs >= 3:
                    v.wait_ge(s_pe, rs - 2)  # one-hot buffer consumed
                lo_src = lo_d[side][:, 16 * r:16 * (r + 1)]
                bs_src = bs_d[side][:, 16 * r:16 * (r + 1)]

                def emit_oh(dig_src, mwidth, coloff, f0, f1):
                    # oh[p, (coloff/CH + m)*CH + f] = (dig[p, f] == iota[m])
                    o = oh[buf][:]
                    v.tensor_tensor(
                        out=bass.AP(o.tensor, o.offset + coloff + f0,
                                    [o.ap[0], [CH, mwidth], [1, f1 - f0]]),
                        in0=bass.AP(dig_src.tensor, dig_src.offset + f0,
                                    [dig_src.ap[0], [0, mwidth], [1, f1 - f0]]),
                        in1=bass.AP(iota_c[:].tensor, coloff + f0,
                                    [iota_c[:].ap[0], [CH, mwidth],
                                     [1, f1 - f0]]),
                        op=mybir.AluOpType.is_equal).then_inc(s_dve, 1)

                def inc_t():
                    nonlocal t
                    t += 1

                HB = NB // 2
                if rs == 0:
                    emit_oh(lo_src, P, 0, 0, CH); inc_t()
                    # iota_b generated in halves; compare each as it lands
                    v.wait_ge(s_pool, 2)
                    emit_oh(bs_src, HB, CH * P, 0, CH); inc_t()
                    v.wait_ge(s_pool, 3)
                    emit_oh(bs_src, NB - HB, CH * (P + HB), 0, CH); inc_t()
                elif rs >= 2 * ROWS - 2:
                    # last two row-sides: halve by chunk so the PE can start
                    # the chain before the second half is built
                    emit_oh(lo_src, P, 0, 0, CH // 2); inc_t()
                    emit_oh(bs_src, NB, CH * P, 0, CH // 2); inc_t()
                    ticks["oh_h1"][rs] = t
                    emit_oh(lo_src, P, 0, CH // 2, CH); inc_t()
                    emit_oh(bs_src, NB, CH * P, CH // 2, CH); inc_t()
                else:
                    emit_oh(lo_src, P, 0, 0, CH); inc_t()
                    emit_oh(bs_src, NB, CH * P, 0, CH); inc_t()
                ticks["oh"][rs] = t
                if rs >= 3 and rs % 2 == 1:
                    tail((rs - 3) // 2)

            tail(ROWS - 1)
            ticks["final"] = t

        @block.scalar
        def _(sc):
            sc.dma_start(out=tok_t[:].rearrange("p (a g) -> p a g", g=16),
                         in_=tok_src(tgt)).then_inc(s_load_t, 16)
            for r in range(ROWS):
                sl = slice(NB * r, NB * (r + 1))
                gp, op_ = g_slot(r, "p")
                gt, ot_ = g_slot(r, "t")
                if r in (0, ROWS - 1):
                    # pred grid is in its own bank: safe right after its chain
                    sc.wait_ge(s_pe, 2 * r + 1)
                else:
                    sc.wait_ge(s_pe, 2 * r + 2)
                sc.activation(pres_p[:, sl], gp[:, op_:op_ + NB],
                              mybir.ActivationFunctionType.Sign,
                              accum_out=res[:, ROWS + r:ROWS + r + 1],
                              ).then_inc(s_act, 1)
                if r < ROWS - 1:
                    sc.wait_ge(s_pe, 2 * r + 2)
                    sc.activation(pres_t[:, sl], gt[:, ot_:ot_ + NB],
                                  mybir.ActivationFunctionType.Sign,
                                  ).then_inc(s_act, 1)
                if r >= 1:
                    sc.wait_ge(s_dve, ticks["prod"][r - 1])
                    sc.activation(junk[:, NB * (r - 1):NB * r],
                                  junk[:, NB * (r - 1):NB * r],
                                  mybir.ActivationFunctionType.Identity,
                                  accum_out=res[:, r - 1:r],
                                  ).then_inc(s_act, 1)

        @block.tensor
        def _(te):
            for rs in range(2 * ROWS):
                r, side = rs // 2, ("p", "t")[rs % 2]
                buf = rs % 3
                if ticks["oh_h1"][rs]:
                    te.wait_ge(s_dve, ticks["oh_h1"][rs])
                else:
                    te.wait_ge(s_dve, ticks["oh"][rs])
                a = oh[buf][:]
                gten, goff = g_slot(r, side)
                for f in range(CH):
                    if ticks["oh_h1"][rs] and f == CH // 2:
                        te.wait_ge(s_dve, ticks["oh"][rs])
                    ins = te.matmul(
                        out=gten[:, goff:goff + NB],
                        lhsT=bass.AP(a.tensor, a.offset + f,
                                     [a.ap[0], [CH, P]]),
                        rhs=bass.AP(a.tensor, a.offset + CH * P + f,
                                    [a.ap[0], [CH, NB]]),
                        start=(f == 0),
                        stop=(f == CH - 1),
                    )
                    if f == CH - 1:
                        ins.then_inc(s_pe, 1)

        @block.sync
        def _(sy):
            sy.dma_start(out=tok_p[:].rearrange("p (a g) -> p a g", g=16),
                         in_=tok_src(pred)).then_inc(s_load_p, 16)
            # rows 0..5 fully reduced once red5 fired (ACT tick 3*6+2=20)
            sy.wait_ge(s_act, 20)
            ra = res[:]
            sy.dma_start(
                out=bass.AP(out[:].tensor, 0, [[16, P], [ROWS, 2], [1, 6]]),
                in_=bass.AP(ra.tensor, 0, [ra.ap[0], [ROWS, 2], [1, 6]]),
            ).then_inc(s_out, 16)
            sy.wait_ge(s_act, 3 * ROWS - 2)
            sy.wait_ge(s_dve, ticks["final"])
            sy.dma_start(
                out=bass.AP(out[:].tensor, 6, [[16, P], [ROWS, 2], [1, 2]]),
                in_=bass.AP(ra.tensor, 6, [ra.ap[0], [ROWS, 2], [1, 2]]),
            ).then_inc(s_out, 16)
            sy.wait_ge(s_out, 32)

    nc.compile()
    return nc


def run(pred_tokens, tgt_tokens, trace=False):
    """Returns (loss, exec_time_ns_or_None)."""
    from concourse.bass_utils import run_bass_kernel_spmd

    if "nc" not in _CACHE:
        _CACHE["nc"] = _build_kernel()
    nc = _CACHE["nc"]

    pred_tokens = np.ascontiguousarray(np.asarray(pred_tokens, dtype=np.int32))
    tgt_tokens = np.ascontiguousarray(np.asarray(tgt_tokens, dtype=np.int32))
    assert pred_tokens.shape == (B, L) and tgt_tokens.shape == (B, L)

    in_maps = [
        {
            "pred": pred_tokens[c * ROWS:(c + 1) * ROWS],
            "tgt": tgt_tokens[c * ROWS:(c + 1) * ROWS],
        }
        for c in range(N_CORES)
    ]
    try:
        kres = run_bass_kernel_spmd(nc, in_maps, list(range(N_CORES)),
                                    trace=trace)
    except ModuleNotFoundError:
        # NTFF profiling hook unavailable in this axon client
        kres = run_bass_kernel_spmd(nc, in_maps, list(range(N_CORES)))

    ov = np.empty(B, dtype=np.float32)
    ss = np.empty(B, dtype=np.float32)
    for c, r in enumerate(kres.results):
        o = r["out"]  # [128, 16] f32: cols 0..7 overlap partials, 8..15 setsize
        ov[c * ROWS:(c + 1) * ROWS] = o[:, :ROWS].sum(axis=0, dtype=np.float64)
        ss[c * ROWS:(c + 1) * ROWS] = o[:, ROWS:].sum(axis=0, dtype=np.float64)

    precision = np.where(ss > 0, ov / np.maximum(ss, np.float32(1.0)),
                         np.float32(0.0)).astype(np.float32)
    loss = np.float32(1.0) - np.float32(precision.mean(dtype=np.float64))
    return loss, kres.exec_time_ns


def kernel(pred_tokens, target_tokens):
    loss, _ = run(pred_tokens, target_tokens)
    return loss


if __name__ == "__main__":
    rng = np.random.default_rng(0)
    p = rng.integers(0, V, (B, L), dtype=np.int32)
    t = rng.integers(0, V, (B, L), dtype=np.int32)
    print(kernel(p, t))



# revision 27
# speedup vs baseline: 6.8385x; 6.8385x over previous
"""BLEU-precision loss kernel for Trainium2 (8 NeuronCores, data parallel).

loss = 1 - mean_i |set(pred_i) & set(tgt_i)| / |set(pred_i)|   over 64 rows.

Per core (8 rows, 16 row-sides), a presence grid over the 32000-id vocab is
built for every row-side by one GPSIMD `dma_scatter_add` (SBUF-destination
parity mode, tokens_per_rank=128): idx = raw token id (int16), payload =
constant 1.0 bf16. Each token lands in cell (partition = t&127, col = t>>8)
of one of two [128,125] half-grids selected by bit 7 of t, so a cell is
nonzero iff the id occurs in the row-side. Indices come from int32->int16
tensor_copy casts (exact for ids < 2^15); only GPSIMD group 0 of the
[128, N] index tile carries real data, the rest is zero-filled once.

Every engine pulls its weight concurrently:
  SP:   three 2-row token loads, final result store.
  ACT:  three 2-row token loads, then |set(pred)| for rows 5..7 via
        Sign+accumulate (a dummy activation absorbs the act-table load
        during its idle window).
  DVE:  payload ones, pred-grid zeros, all eight index casts, |set(pred)|
        for rows 0..4 (is_gt+accum), overlap rows 0..3 (logical_and then
        mult+accum).
  Pool: two 2-row loads, index-tile fill, tgt-grid zeros, the 16 scatters,
        overlap rows 4..7.

Host sums the [128,16] f32 per-partition partials and finishes the exact
precision / mean arithmetic.
"""
import sys

sys.path.insert(0, "/opt/trn_rl_repo")

import numpy as np
import concourse.bass as bass
import concourse.bacc as bacc
from concourse import mybir

B = 64          # total rows
ROWS = 8        # rows per core
N_CORES = 8
L = 2048        # tokens per row
V = 32000       # vocab
P = 128
F32 = mybir.dt.float32
BF16 = mybir.dt.bfloat16
I32 = mybir.dt.int32
I16 = mybir.dt.int16

# scatter order: all pred rows, then tgt rows 4..7 (whose casts complete
# early via the Pool-loaded halves), then tgt rows 0..3
SC_SIDES = [(r, 0) for r in range(ROWS)] + \
    [(4, 1), (5, 1), (0, 1), (1, 1), (2, 1), (3, 1), (6, 1), (7, 1)]
SC_J = {rs: j for j, rs in enumerate(SC_SIDES)}
NSC = len(SC_SIDES)  # 16

DVE_SS_ROWS = (0, 1, 2, 3, 4, 5, 6, 7)
DVE_OV_ROWS = (4, 5, 0, 1, 2, 3, 6, 7)
DVE_OV_AND_ONLY = ()
POOL_OV_ACC_ONLY = ()
POOL_OV_ROWS = ()

_CACHE = {}


def _grid_blk(row, side):
    return 2 * row + side


def _build_kernel():
    nc = bacc.Bacc()
    pred = nc.dram_tensor("pred", [ROWS, L], I32, kind="ExternalInput")
    tgt = nc.dram_tensor("tgt", [ROWS, L], I32, kind="ExternalInput")
    out = nc.dram_tensor("out", [P, 16], F32, kind="ExternalOutput")

    from contextlib import ExitStack

    es = ExitStack()
    with es:
        sb = lambda name, shape, dt: es.enter_context(nc.sbuf_tensor(name, shape, dt))
        sem = lambda name: es.enter_context(nc.semaphore(name))

        tok32 = sb("tok32", [16, 2048], I32)     # pred cols 0:1024, tgt 1024:2048
        idx16 = sb("idx16", [P, NSC * 128], I16)
        ones = sb("ones", [P, 16], BF16)
        grid = sb("grid", [P, NSC * 256], BF16)
        junkV = sb("junkV", [P, ROWS * 256], BF16)
        junkO = sb("junkO", [P, ROWS * 256], BF16)
        junkS = sb("junkS", [P, ROWS * 256], BF16)
        res = sb("res", [P, 16], F32)

        # per-2-row load/cast sems: pred halves a..d, tgt halves a..d
        s_ld = {k: sem(f"s_ld_{k}") for k in
                ("pa", "pb", "pc", "pd", "ta", "tb", "tc", "td")}
        s_cast = {k: sem(f"s_cast_{k}") for k in
                  ("pa", "pb", "pc", "pd", "ta", "tb", "tc", "td")}
        s_jf = sem("s_jf")
        s_zp = sem("s_zp")       # DVE: ones + pred-grid zeros
        s_zt = sem("s_zt")       # Pool: tgt-grid zeros
        s_dve = sem("s_dve")
        s_ss = sem("s_ss")       # ACT setsize Signs
        s_povl = sem("s_povl")   # Pool overlap tail ops
        s_sc = [sem(f"s_sc{j}") for j in range(NSC)]
        s_out = sem("s_out")

        block = es.enter_context(nc.Block())

        ticks = {"final": 0}

        def seg2(t, blk):
            a = t[:]
            return bass.AP(a.tensor, a.offset + blk * 256,
                           [a.ap[0], [128, 2], [1, 125]])

        def zeros_ap(side):
            a = grid[:].bitcast(I32)
            return bass.AP(a.tensor, a.offset + side * 128,
                           [a.ap[0], [256, ROWS], [1, 128]])

        def ld2(eng, side_t, coloff, rows0, s):
            # load rows0..rows0+2 of one side into tok32
            eng.dma_start(
                out=bass.AP(tok32[:].tensor, coloff + rows0 * 128,
                            [[2048, 16], [128, 2], [1, 128]]),
                in_=bass.AP(side_t[:].tensor, rows0 * L,
                            [[128, 16], [L, 2], [1, 128]]),
            ).then_inc(s, 16)

        # (key, side, rows0): cast granules; tok col = side*1024 + rows0*128
        GRAN = [("pa", 0, 0), ("pb", 0, 2), ("pc", 0, 4), ("pd", 0, 6),
                ("ta", 1, 0), ("tb", 1, 2), ("tc", 1, 4), ("td", 1, 6)]
        GKEY = {(s, r0): k for k, s, r0 in GRAN}

        def cast_key(row, side):
            return GKEY[(side, (row // 2) * 2)]

        # ---------------- DVE ----------------
        @block.vector
        def _(v):
            t = 0

            def inc(ins):
                nonlocal t
                t += 1
                return ins.then_inc(s_dve, 1)

            v.memset(ones[:], 1.0).then_inc(s_zp, 1)
            v.memset(zeros_ap(0), 0).then_inc(s_zp, 1)
            # index casts in 2-row granules, ordered by load-sem arrival:
            # pa/pc first (ACT+SP first loads), then pb/pd, tgt c/d (Pool
            # loads), then tgt a/b
            v.wait_ge(s_jf, 1)
            for k in ("pa", "pc", "pb", "pd", "tc", "ta", "tb", "td"):
                side = 0 if k[0] == "p" else 1
                r0 = {"a": 0, "b": 2, "c": 4, "d": 6}[k[1]]
                c0 = side * 1024 + r0 * 128
                v.wait_ge(s_ld[k], 16)
                v.tensor_copy(idx16[:16, c0:c0 + 256],
                              tok32[:, c0:c0 + 256]).then_inc(s_cast[k], 1)

            for r in DVE_SS_ROWS:
                v.wait_ge(s_sc[SC_J[(r, 0)]], 16)
                inc(v.tensor_scalar(out=seg2(junkS, r),
                                    in0=seg2(grid, _grid_blk(r, 0)),
                                    scalar1=0.5, scalar2=None,
                                    op0=mybir.AluOpType.is_gt,
                                    op1=mybir.AluOpType.add,
                                    accum_out=res[:, 8 + r:9 + r]))
            for r in DVE_OV_ROWS:
                v.wait_ge(s_sc[SC_J[(r, 0)]], 16)
                v.wait_ge(s_sc[SC_J[(r, 1)]], 16)
                inc(v.tensor_tensor(out=seg2(junkV, r),
                                    in0=seg2(grid, _grid_blk(r, 0)),
                                    in1=seg2(grid, _grid_blk(r, 1)),
                                    op=mybir.AluOpType.logical_and))
                v.wait_ge(s_dve, t)
                inc(v.tensor_scalar(out=seg2(junkO, r),
                                    in0=seg2(junkV, r),
                                    scalar1=1.0, scalar2=None,
                                    op0=mybir.AluOpType.mult,
                                    op1=mybir.AluOpType.add,
                                    accum_out=res[:, r:r + 1]))
            for r in DVE_OV_AND_ONLY:
                v.wait_ge(s_sc[SC_J[(r, 0)]], 16)
                v.wait_ge(s_sc[SC_J[(r, 1)]], 16)
                inc(v.tensor_tensor(out=seg2(junkV, r),
                                    in0=seg2(grid, _grid_blk(r, 0)),
                                    in1=seg2(grid, _grid_blk(r, 1)),
                                    op=mybir.AluOpType.logical_and))
            ticks["and2"] = t
            ticks["final"] = t

        # ---------------- Pool ----------------
        @block.gpsimd
        def _(g):
            g.memset(idx16[:].bitcast(I32), 0).then_inc(s_jf, 1)
            ld2(g, tgt, 1024, 4, s_ld["tc"])
            ld2(g, tgt, 1024, 6, s_ld["td"])
            g.memset(zeros_ap(1), 0).then_inc(s_zt, 1)
            waited = set()
            for j, (r, s) in enumerate(SC_SIDES):
                if j == 0:
                    g.wait_ge(s_zp, 2)
                if s == 1 and "zt" not in waited:
                    waited.add("zt")
                    g.wait_ge(s_zt, 1)
                ck = cast_key(r, s)
                if ck not in waited:
                    waited.add(ck)
                    g.wait_ge(s_cast[ck], 1)
                blk = _grid_blk(r, s)
                ic = (s * 8 + r) * 128
                g.dma_scatter_add(
                    out_ap=grid[:, blk * 256:blk * 256 + 125],
                    out_ap_other=grid[:, blk * 256 + 128:blk * 256 + 253],
                    in_ap=ones[:].rearrange("p (s e) -> p s e", e=1),
                    idxs_ap=idx16[:, ic:ic + 128],
                    num_idxs=L,
                    num_idxs_reg=L,
                    elem_size=1,
                    sbuf_tokens_per_rank=128,
                    parity_reg=0,
                ).then_inc(s_sc[j], 16)
            npovl = 0
            for r in POOL_OV_ACC_ONLY:
                g.wait_ge(s_dve, ticks["and2"])
                g.tensor_scalar(out=seg2(junkO, r),
                                in0=seg2(junkV, r),
                                scalar1=1.0, scalar2=None,
                                op0=mybir.AluOpType.mult,
                                op1=mybir.AluOpType.add,
                                accum_out=res[:, r:r + 1]).then_inc(s_povl, 1)
                npovl += 1
            for r in POOL_OV_ROWS:
                g.wait_ge(s_sc[SC_J[(r, 0)]], 16)
                g.wait_ge(s_sc[SC_J[(r, 1)]], 16)
                g.tensor_tensor(out=seg2(junkV, r),
                                in0=seg2(grid, _grid_blk(r, 0)),
                                in1=seg2(grid, _grid_blk(r, 1)),
                                op=mybir.AluOpType.logical_and).then_inc(s_povl, 1)
                npovl += 1
                g.wait_ge(s_povl, npovl)
                g.tensor_scalar(out=seg2(junkO, r),
                                in0=seg2(junkV, r),
                                scalar1=1.0, scalar2=None,
                                op0=mybir.AluOpType.mult,
                                op1=mybir.AluOpType.add,
                                accum_out=res[:, r:r + 1]).then_inc(s_povl, 1)
                npovl += 1
            ticks["povl"] = npovl

        # ---------------- ACT ----------------
        @block.scalar
        def _(sc):
            ld2(sc, pred, 0, 0, s_ld["pa"])
            ld2(sc, pred, 0, 2, s_ld["pb"])
            ld2(sc, tgt, 1024, 0, s_ld["ta"])

        # ---------------- SP ----------------
        @block.sync
        def _(sy):
            ld2(sy, pred, 0, 4, s_ld["pc"])
            ld2(sy, pred, 0, 6, s_ld["pd"])
            ld2(sy, tgt, 1024, 2, s_ld["tb"])
            sy.wait_ge(s_dve, ticks["final"])
            sy.wait_ge(s_povl, ticks["povl"])
            sy.dma_start(out=out[:], in_=res[:]).then_inc(s_out, 16)
            sy.wait_ge(s_out, 16)

    nc.compile()
    return nc


def run(pred_tokens, tgt_tokens, trace=False):
    """Returns (loss, exec_time_ns_or_None)."""
    from concourse.bass_utils import run_bass_kernel_spmd

    if "nc" not in _CACHE:
        _CACHE["nc"] = _build_kernel()
    nc = _CACHE["nc"]

    pred_tokens = np.ascontiguousarray(np.asarray(pred_tokens, dtype=np.int32))
    tgt_tokens = np.ascontiguousarray(np.asarray(tgt_tokens, dtype=np.int32))
    assert pred_tokens.shape == (B, L) and tgt_tokens.shape == (B, L)

    in_maps = [
        {
            "pred": pred_tokens[c * ROWS:(c + 1) * ROWS],
            "tgt": tgt_tokens[c * ROWS:(c + 1) * ROWS],
        }
        for c in range(N_CORES)
    ]
    try:
        kres = run_bass_kernel_spmd(nc, in_maps, list(range(N_CORES)),
                                    trace=trace)
    except ModuleNotFoundError:
        kres = run_bass_kernel_spmd(nc, in_maps, list(range(N_CORES)))

    ov = np.empty(B, dtype=np.float64)
    ss = np.empty(B, dtype=np.float64)
    for c, r in enumerate(kres.results):
        o = r["out"]  # [128, 16] f32: cols 0..8 overlap, 8..16 setsize
        ov[c * ROWS:(c + 1) * ROWS] = o[:, :ROWS].sum(axis=0, dtype=np.float64)
        ss[c * ROWS:(c + 1) * ROWS] = o[:, ROWS:].sum(axis=0, dtype=np.float64)

    precision = np.where(ss > 0, ov / np.maximum(ss, 1.0), 0.0)
    loss = np.float32(1.0) - np.float32(precision.mean())
    return loss, kres.exec_time_ns


def kernel(pred_tokens, target_tokens):
    loss, _ = run(pred_tokens, target_tokens)
    return loss


if __name__ == "__main__":
    rng = np.random.default_rng(0)
    p = rng.integers(0, V, (B, L), dtype=np.int32)
    t = rng.integers(0, V, (B, L), dtype=np.int32)
    print(kernel(p, t))
